# revision 15
# baseline (speedup 1.0000x reference)
"""GAT layer (nn_GAT_57543971832576) Bass/Tile kernel for 8 Trainium2 NeuronCores.

Math (reference):
    x' = x @ W + bias
    S_ij = leaky_relu(f1_i + f2_j, 0.2),  f1 = x'@phi1, f2 = x'@phi2
    A = softmax_j(where(adj+I > 0, S, -1e9))
    h = A @ x'

Reformulation used on device (core owns rows i = [c*1024, (c+1)*1024)):
    leaky_relu(s, 0.2) = 0.6*s + 0.4*|s|; softmax rows are invariant to the
    per-row shift 0.6*f1_i, so the masked attention numerator is
        B'_ij = adj_ij * exp(0.4*|f1_i + f2_j| + 0.6*f2_j)
    and, using sum_j A_ij = 1 to pull bias out of the values,
        P[k, i]  = sum_j x[j, k] * B'^T[j, i]      (PE, lhsT = x tiles)
        den[i]   = sum_j B'^T[j, i]                (PE, lhsT = ones)
        hT[f, i] = W^T @ (P + x_i^T * q_i) ;  h_i = hT[:, i]/(den_i + q_i) + b
    where q_i = (1 - adj_ii) * exp(0.4*|s_ii| + 0.6*f2_i) is the forced
    self-loop term for rows without one.

The host ships adj^T per core (pre-swizzled [p, t, i] f16 -- exact for a 0/1
matrix) so the device never transposes the big matrix; the j-contraction runs
directly over [j-partition, i-free] tiles.
"""

import numpy as np

N = 8192
F = 128
NCORES = 8
R = N // NCORES          # rows per core (1024)
TJ = N // 128            # j-tiles (64)
ISUB = R // 128          # core-row subtiles (8)
CH = 8                   # adj chunks
TPC = TJ // CH           # j-tiles per chunk (8)

_CACHE = {}
TRACE = False            # set True (e.g. from test.py) to capture an NTFF profile
LAST_EXEC_NS = None      # exec time from the last traced run
LAST_RESULTS = None      # full BassKernelResults of the last run


def _build_nc():
    import concourse.bass as bass
    import concourse.mybir as mybir
    import concourse.tile as tile
    from concourse import masks

    f32 = mybir.dt.float32
    f16 = mybir.dt.float16
    i16 = mybir.dt.int16
    i32 = mybir.dt.int32
    Alu = mybir.AluOpType
    Act = mybir.ActivationFunctionType

    nc = bass.Bass("TRN2", target_bir_lowering=False, debug=False,
                   num_devices=NCORES)

    adjs_d = nc.dram_tensor("adjs", [128, TJ, R], f16, kind="ExternalInput").ap()
    x16n_d = nc.dram_tensor("x16n", [128, TJ, F], f16, kind="ExternalInput").ap()
    xT16_d = nc.dram_tensor("xT16", [128, TJ, 128], f16, kind="ExternalInput").ap()
    xcT_d = nc.dram_tensor("xcT16", [128, ISUB, 128], f16, kind="ExternalInput").ap()
    W_d = nc.dram_tensor("weight", [128, 128], f32, kind="ExternalInput").ap()
    bp_d = nc.dram_tensor("bp", [128, 4], f32, kind="ExternalInput").ap()
    adjd_d = nc.dram_tensor("adjd", [128, ISUB], f32, kind="ExternalInput").ap()
    out_d = nc.dram_tensor("out", [R, F], f32, kind="ExternalOutput").ap()

    with tile.TileContext(nc) as tc:
        with tc.tile_pool(name="const", bufs=1) as cp, \
             tc.tile_pool(name="mmps", bufs=1, space="PSUM") as mmps, \
             tc.tile_pool(name="ppsA", bufs=1, space="PSUM") as ppsA, \
             tc.tile_pool(name="ppsB", bufs=2, space="PSUM") as ppsB:
            # ---------------- psum layout (8 banks exactly) ----------------
            P0 = mmps.tile([128, 512], f32, name="P0")
            P1 = mmps.tile([128, 512], f32, name="P1")
            d0 = mmps.tile([1, 512], f32, name="d0")
            d1 = mmps.tile([1, 512], f32, name="d1")
            smA = ppsA.tile([128, 512], f32, name="smA")
            smB = ppsA.tile([128, 160], f32, name="smB")

            def bc_ps():
                return ppsB.tile([128, 512], f32, tag="bc", name="bc")

            # ---------------- constants + inputs ----------------
            ones = cp.tile([1, 128], f32)
            nc.vector.memset(ones[:], 1.0)
            ones16 = cp.tile([128, 1], f16)
            nc.vector.memset(ones16[:], 1.0)
            ident = cp.tile([128, 128], f32)
            masks.make_identity(nc, ident[:])
            scr = cp.tile([1, 16], f32)     # DVE scratch for DMA-lane touches

            W_sb = cp.tile([128, 128], f32)
            nc.sync.dma_start(W_sb[:], W_d)
            bp = cp.tile([128, 4], f32)
            nc.sync.dma_start(bp[:], bp_d)
            adjd = cp.tile([128, ISUB], f32)
            nc.sync.dma_start(adjd[:], adjd_d)
            xT16 = cp.tile([128, TJ, 128], f16)
            nc.sync.dma_start(xT16[:], xT16_d)
            x16 = cp.tile([128, TJ, F], f16)
            nc.sync.dma_start(x16[:], x16n_d)
            xcT = cp.tile([128, ISUB, 128], f16)
            nc.sync.dma_start(xcT[:], xcT_d)

            # Observation chain: one tiny PE matmul reading each DMA-loaded
            # tensor, WAW-chained into one psum slot.  Later PE instructions
            # get a free same-engine ordering edge, so no matmul carries more
            # than one cross-engine sync-wait (walrus LDW fits one wait).
            obs = smB[0:1, 144:145]
            ob = nc.tensor.matmul(obs, ident[:, 0:1], ident[:, 0:1],
                                  start=True, stop=True)
            for src in (W_sb[:, 0:1], bp[:, 0:1], adjd[:, 0:1],
                        xT16[:, 0, 0:1], x16[:, 0, 0:1], xcT[:, 0, 0:1]):
                nxt = nc.tensor.matmul(obs, src, src, start=True, stop=True)
                tile.add_dep_helper(nxt.ins, ob.ins, sync=False, reason="obs")
                ob = nxt

            def _pe(inst):
                tile.add_dep_helper(inst.ins, ob.ins, sync=False, reason="obs")
                return inst

            # ---------------- prep ----------------
            # W^T (for Wphi), then Wphi = W @ phi so f = x @ Wphi
            _pe(nc.tensor.transpose(smA[:, 0:128], W_sb[:], ident[:]))
            WT = cp.tile([128, 128], f32)
            nc.scalar.copy(WT[:], smA[:, 0:128])
            _pe(nc.tensor.matmul(smA[:, 128:130], WT[:], bp[:, 1:3],
                                 start=True, stop=True))
            Wphi16 = cp.tile([128, 2], f16)
            nc.scalar.copy(Wphi16[:], smA[:, 128:130])

            # c1 = b@phi1, c2 = b@phi2 (bias-fold constants), broadcasts
            _pe(nc.tensor.matmul(smA[0:1, 132:134], bp[:, 0:1], bp[:, 1:3],
                                 start=True, stop=True))
            crow = cp.tile([1, 2], f32)
            nc.scalar.copy(crow[:], smA[0:1, 132:134])
            c12 = cp.tile([1, 1], f32)
            nc.vector.tensor_tensor(c12[:], crow[0:1, 0:1], crow[0:1, 1:2],
                                    op=Alu.add)
            _pe(nc.tensor.matmul(smA[:, 136:137], ones[:], c12[:],
                                 start=True, stop=True))
            c12b = cp.tile([128, 1], f32)
            nc.scalar.copy(c12b[:], smA[:, 136:137])
            _pe(nc.tensor.matmul(smA[:, 140:141], ones[:], crow[0:1, 1:2],
                                 start=True, stop=True))
            c2b06 = cp.tile([128, 1], f32)
            nc.scalar.activation(c2b06[:], smA[:, 140:141], Act.Copy, scale=0.6)
            bias2 = cp.tile([128, 1], f32)
            nc.scalar.copy(bias2[:], bp[:, 0:1])

            # f1/f2 projections, column form: smB[:, 2t:2t+2] = x_tile @ Wphi
            for t in range(TJ):
                _pe(nc.tensor.matmul(smB[:, 2 * t:2 * t + 2], xT16[:, t, :],
                                     Wphi16[:], start=True, stop=True))
            # core-row copies (program is SPMD-identical; core rows come from
            # the per-core xcT input): smB[:, 128+2tt : 128+2tt+2]
            for t in range(ISUB):
                _pe(nc.tensor.matmul(smB[:, 128 + 2 * t:128 + 2 * t + 2],
                                     xcT[:, t, :], Wphi16[:],
                                     start=True, stop=True))
            f2c = cp.tile([128, TJ], f32)
            nc.scalar.copy(f2c[:], smB[:, 1:129:2])
            wb = cp.tile([128, TJ], f32)    # ACT bias: 0.6*f2_j (= log w_j)
            nc.scalar.activation(wb[:], smB[:, 1:129:2], Act.Identity,
                                 scale=0.6, bias=c2b06[:])

            # f1 at core rows, row form -> broadcast over partitions (+c1+c2)
            f1r_ps = (bc_ps(), bc_ps())
            for g in range(2):
                _pe(nc.tensor.matmul(
                    f1r_ps[g][0:1, :], Wphi16[:, 0:1],
                    xcT[:, 4 * g:4 * g + 4, :].rearrange("p a b -> p (a b)"),
                    start=True, stop=True))
            f1row = cp.tile([1, R], f32)
            nc.scalar.copy(f1row[0:1, 0:512], f1r_ps[0][0:1, :])
            nc.scalar.copy(f1row[0:1, 512:1024], f1r_ps[1][0:1, :])
            nc.vector.memset(f1r_ps[0][0:1, 0:1], 0.0)   # DVE touch: WAR absorber
            nc.vector.memset(f1r_ps[1][0:1, 0:1], 0.0)
            f1b_ps = (bc_ps(), bc_ps())
            for g in range(2):
                _pe(nc.tensor.matmul(f1b_ps[g][:], ones[:],
                                     f1row[0:1, 512 * g:512 * (g + 1)],
                                     start=True, stop=True))
            f1b = cp.tile([128, R], f16)
            nc.scalar.activation(f1b[:, 0:512], f1b_ps[0][:], Act.Identity,
                                 bias=c12b[:])
            nc.scalar.activation(f1b[:, 512:1024], f1b_ps[1][:], Act.Identity,
                                 bias=c12b[:])
            nc.vector.memset(f1b_ps[0][0:1, 0:1], 0.0)
            nc.vector.memset(f1b_ps[1][0:1, 0:1], 0.0)

            # diagonal (self-loop) correction q, column form then row form
            fcore = cp.tile([128, 2 * ISUB], f32)
            nc.scalar.copy(fcore[:], smB[:, 128:144])
            sd = cp.tile([128, ISUB], f32)
            nc.vector.tensor_tensor(sd[:], fcore[:, 0:16:2], fcore[:, 1:16:2],
                                    op=Alu.add)
            sd2 = cp.tile([128, ISUB], f32)
            nc.vector.tensor_scalar(sd2[:], sd[:], c12b[:], None, op0=Alu.add)
            absd = cp.tile([128, ISUB], f32)
            nc.vector.tensor_scalar(absd[:].bitcast(i32), sd2[:].bitcast(i32),
                                    0x7FFFFFFF, None, op0=Alu.bitwise_and)
            Gd = cp.tile([128, ISUB], f32)
            nc.scalar.activation(Gd[:], absd[:], Act.Exp, scale=0.4)
            wd = cp.tile([128, ISUB], f32)
            nc.scalar.activation(wd[:], fcore[:, 1:16:2], Act.Exp,
                                 scale=0.6, bias=c2b06[:])
            mneg = cp.tile([128, ISUB], f32)
            nc.scalar.activation(mneg[:], adjd[:], Act.Identity,
                                 scale=-1.0, bias=1.0)
            qGw = cp.tile([128, ISUB], f32)
            nc.vector.tensor_tensor(qGw[:], Gd[:], wd[:], op=Alu.mult)
            qn = cp.tile([128, ISUB], f32)
            nc.vector.tensor_tensor(qn[:], qGw[:], mneg[:], op=Alu.mult)

            q_ps = (bc_ps(), bc_ps())
            for t in range(ISUB):
                _pe(nc.tensor.matmul(q_ps[t // 4][0:1, 128 * (t % 4):128 * (t % 4) + 128],
                                     qn[:, t:t + 1], ident[:],
                                     start=True, stop=True))
            q_row = cp.tile([1, R], f32)
            nc.scalar.copy(q_row[0:1, 0:512], q_ps[0][0:1, :])
            nc.scalar.copy(q_row[0:1, 512:1024], q_ps[1][0:1, :])
            nc.vector.memset(q_ps[0][0:1, 0:1], 0.0)
            nc.vector.memset(q_ps[1][0:1, 0:1], 0.0)
            qb_ps = (bc_ps(), bc_ps())
            for g in range(2):
                _pe(nc.tensor.matmul(qb_ps[g][:], ones[:],
                                     q_row[0:1, 512 * g:512 * (g + 1)],
                                     start=True, stop=True))
            qb16 = cp.tile([128, R], f16)
            nc.scalar.copy(qb16[:, 0:512], qb_ps[0][:])
            nc.scalar.copy(qb16[:, 512:1024], qb_ps[1][:])
            nc.vector.memset(qb_ps[0][0:1, 0:1], 0.0)
            nc.vector.memset(qb_ps[1][0:1, 0:1], 0.0)

            # ---------------- main loop ----------------
            with tc.tile_pool(name="adjp", bufs=3) as adjp, \
                 tc.tile_pool(name="absp", bufs=3) as absp, \
                 tc.tile_pool(name="gp", bufs=3) as gp, \
                 tc.tile_pool(name="btp", bufs=4) as btp:
                for ch in range(CH):
                    adjq = adjp.tile([128, TPC, R], f16, tag="adj")
                    nc.sync.dma_start(adjq[:], adjs_d[:, ch * TPC:(ch + 1) * TPC, :])
                    # DVE observes this chunk's DMA once, so the mults below
                    # don't each carry a DMA-lane wait.
                    nc.vector.tensor_copy(scr[0:1, ch:ch + 1],
                                          adjq[0:1, 0, 0:1])
                    for tt in range(TPC):
                        t = ch * TPC + tt
                        first = t == 0
                        last = t == TJ - 1
                        s16 = absp.tile([128, R], f16, tag="s")
                        nc.vector.tensor_scalar(s16[:], f1b[:],
                                                f2c[:, t:t + 1], None,
                                                op0=Alu.add)
                        absS = absp.tile([128, R], f16, tag="abs")
                        nc.vector.tensor_scalar(absS[:].bitcast(i16),
                                                s16[:].bitcast(i16),
                                                0x7FFF, None,
                                                op0=Alu.bitwise_and)
                        g16 = gp.tile([128, R], f16, tag="g")
                        nc.scalar.activation(g16[:], absS[:], Act.Exp,
                                             scale=0.4, bias=wb[:, t:t + 1])
                        bt = btp.tile([128, R], f16, tag="bt")
                        # mask multiply: ~2/3 on GpSimd (otherwise idle), rest
                        # on DVE, which paces the loop
                        eng = nc.vector if t % 3 == 0 else nc.gpsimd
                        eng.tensor_tensor(bt[:], g16[:], adjq[:, tt, :],
                                          op=Alu.mult)
                        mm = nc.tensor.matmul(P0[:], x16[:, t, :], bt[:, 0:512],
                                              start=first, stop=last)
                        if first:
                            tile.add_dep_helper(mm.ins, ob.ins, sync=False,
                                                reason="obs")
                        nc.tensor.matmul(P1[:], x16[:, t, :], bt[:, 512:1024],
                                         start=first, stop=last)
                        nc.tensor.matmul(d0[:], ones16[:], bt[:, 0:512],
                                         start=first, stop=last)
                        nc.tensor.matmul(d1[:], ones16[:], bt[:, 512:1024],
                                         start=first, stop=last)

            # ---------------- epilogue ----------------
            with tc.tile_pool(name="ep", bufs=1) as ep:
                # den + q, reciprocal broadcast over partitions
                dt = ep.tile([1, R], f32)
                nc.vector.tensor_tensor(dt[0:1, 0:512], d0[:],
                                        q_row[0:1, 0:512], op=Alu.add)
                nc.vector.tensor_tensor(dt[0:1, 512:1024], d1[:],
                                        q_row[0:1, 512:1024], op=Alu.add)
                recb = ep.tile([128, R], f32)
                rec_in = ep.tile([128, R], f32)
                rscr = ep.tile([128, R], f32)
                dtb_ps = (bc_ps(), bc_ps())
                for g in range(2):
                    _pe(nc.tensor.matmul(dtb_ps[g][:], ones[:],
                                         dt[0:1, 512 * g:512 * (g + 1)],
                                         start=True, stop=True))
                    nc.vector.tensor_copy(rec_in[:, 512 * g:512 * (g + 1)],
                                          dtb_ps[g][:])
                nc.vector.reciprocal_approx_accurate(recb[:], rec_in[:],
                                                     rscr[:])

                # numerator self-loop correction in P-space, then hT = W^T @ Pc
                u = ep.tile([128, R], f32)
                nc.vector.tensor_tensor(u[:], xcT[:].rearrange("p a b -> p (a b)"),
                                        qb16[:], op=Alu.mult)
                Pc = ep.tile([128, R], f32)
                nc.vector.tensor_tensor(Pc[:, 0:512], P0[:], u[:, 0:512],
                                        op=Alu.add)
                nc.vector.tensor_tensor(Pc[:, 512:1024], P1[:], u[:, 512:1024],
                                        op=Alu.add)
                hT_ps = (bc_ps(), bc_ps())
                for g in range(2):
                    _pe(nc.tensor.matmul(hT_ps[g][:], W_sb[:],
                                         Pc[:, 512 * g:512 * (g + 1)],
                                         start=True, stop=True))
                hn = ep.tile([128, R], f32)
                nc.vector.tensor_tensor(hn[:, 0:512], hT_ps[0][:],
                                        recb[:, 0:512], op=Alu.mult)
                nc.vector.tensor_tensor(hn[:, 512:1024], hT_ps[1][:],
                                        recb[:, 512:1024], op=Alu.mult)
                ho = ep.tile([128, R], f32)
                nc.scalar.activation(ho[:], hn[:], Act.Identity, bias=bias2[:])

                # transpose [f, i] -> [i, f] and store
                hout = ep.tile([128, ISUB, 128], f32)
                for t in range(ISUB):
                    sl = smA[:, 128 * (t % 4):128 * (t % 4) + 128]
                    _pe(nc.tensor.transpose(sl, ho[:, 128 * t:128 * (t + 1)],
                                            ident[:]))
                    nc.vector.tensor_copy(hout[:, t, :], sl)
                nc.sync.dma_start(out_d.rearrange("(a p) f -> p a f", p=128),
                                  hout[:])

    # Walrus fits at most one sync-wait per instruction; Tile emits more.
    # Run bacc's splitter (extra waits move onto EventSemaphore insts).
    from concourse.bass import _bass_rust
    _bass_rust.generate_event_semaphores(nc)
    return nc


def kernel(adj, input, weight, bias, phi):
    """Full inputs in, full output out. Shards row-wise across 8 NeuronCores."""
    adj = np.ascontiguousarray(np.asarray(adj, dtype=np.float32))
    x = np.ascontiguousarray(np.asarray(input, dtype=np.float32))
    W = np.ascontiguousarray(np.asarray(weight, dtype=np.float32))
    b = np.ascontiguousarray(np.asarray(bias, dtype=np.float32))
    phi = np.ascontiguousarray(np.asarray(phi, dtype=np.float32))

    if not _CACHE.get("use_fallback"):
        try:
            return _kernel_bass(adj, x, W, b, phi)
        except Exception:
            import traceback
            traceback.print_exc()
            _CACHE["use_fallback"] = True
    return _kernel_jax_fallback(adj, x, W, b, phi)


def _kernel_bass(adj, x, W, b, phi):
    from concourse.bass_utils import run_bass_kernel_spmd

    if "nc" not in _CACHE:
        _CACHE["nc"] = _build_nc()
    nc = _CACHE["nc"]

    # adjs[c, p, t, il] = adj[c*R + il, t*128 + p]  (transposed + swizzled, f16)
    adjs = np.ascontiguousarray(
        adj.reshape(NCORES, R, TJ, 128).transpose(0, 3, 2, 1)).astype(np.float16)
    xT = np.ascontiguousarray(x.T).astype(np.float16)          # [128, 8192]
    x16n = np.ascontiguousarray(
        x.reshape(TJ, 128, F).transpose(1, 0, 2)).astype(np.float16)
    diag = np.ascontiguousarray(np.diagonal(adj)).astype(np.float32)
    bp = np.ascontiguousarray(
        np.stack([b, phi[:F, 0], phi[F:, 0], np.zeros_like(b)], axis=1)
    ).astype(np.float32)

    in_maps = []
    for c in range(NCORES):
        r0 = c * R
        in_maps.append({
            "adjs": adjs[c],
            "x16n": x16n,
            "xT16": xT.reshape(128, TJ, 128),
            "xcT16": np.ascontiguousarray(
                xT[:, r0:r0 + R]).reshape(128, ISUB, 128),
            "weight": W,
            "bp": bp,
            "adjd": np.ascontiguousarray(
                diag[r0:r0 + R].reshape(ISUB, 128).T),
        })

    res = run_bass_kernel_spmd(nc, in_maps, core_ids=list(range(NCORES)),
                               trace=TRACE)
    global LAST_EXEC_NS, LAST_RESULTS
    LAST_RESULTS = res
    LAST_EXEC_NS = res.exec_time_ns
    parts = [res.results[c]["out"] for c in range(NCORES)]
    return np.concatenate(parts, axis=0).astype(np.float32)


def _kernel_jax_fallback(adj, x, W, b, phi):
    """Device fallback (sharded jax on the 8 NeuronCores) if the Bass path
    fails to compile/run in this environment."""
    import jax
    import jax.numpy as jnp
    from jax import lax
    from jax.sharding import Mesh, PartitionSpec, NamedSharding

    devs = jax.devices()[:NCORES]
    mesh = Mesh(np.asarray(devs), ("i",))
    row = NamedSharding(mesh, PartitionSpec("i", None))
    rep = NamedSharding(mesh, PartitionSpec())

    @jax.jit
    def f(adj_s, x_r, W_r, b_r, phi_r):
        xp = x_r @ W_r + b_r
        f1 = xp @ phi_r[:F]                      # [N, 1]
        f2 = xp @ phi_r[F:]                      # [N, 1]
        w = jnp.exp(jnp.float32(0.6) * f2[:, 0])  # [N]
        ri = lax.broadcasted_iota(jnp.int32, (N, N), 0)
        ci = lax.broadcasted_iota(jnp.int32, (N, N), 1)
        m = (adj_s > 0) | (ri == ci)
        G = jnp.exp(jnp.float32(0.4) * jnp.abs(f1 + f2.T))
        B = jnp.where(m, G * w[None, :], jnp.float32(0.0)).astype(jnp.float16)
        xpa = jnp.concatenate([xp, jnp.ones((N, 1), jnp.float32)],
                              axis=1).astype(jnp.float16)
        num = (B @ xpa).astype(jnp.float32)      # [N/8, F+1]
        return num[:, :F] / num[:, F:F + 1]

    args = (jax.device_put(adj, row), jax.device_put(x, rep),
            jax.device_put(W, rep), jax.device_put(b, rep),
            jax.device_put(phi, rep))
    out = f(*args)
    out.block_until_ready()
    if TRACE:
        import time
        global LAST_EXEC_NS
        reps = 5
        t0 = time.perf_counter()
        for _ in range(reps):
            out = f(*args)
        out.block_until_ready()
        LAST_EXEC_NS = int((time.perf_counter() - t0) / reps * 1e9)
    return np.asarray(out).astype(np.float32)


# revision 16
# speedup vs baseline: 62.5906x; 62.5906x over previous
"""GAT layer (nn_GAT_57543971832576) Bass/Tile kernel for 8 Trainium2 NeuronCores.

Math (reference):
    x' = x @ W + bias
    S_ij = leaky_relu(f1_i + f2_j, 0.2),  f1 = x'@phi1, f2 = x'@phi2
    A = softmax_j(where(adj+I > 0, S, -1e9))
    h = A @ x'

Reformulation used on device (core owns rows i = [c*1024, (c+1)*1024)):
    leaky_relu(s, 0.2) = 0.6*s + 0.4*|s|; softmax rows are invariant to the
    per-row shift 0.6*f1_i, so the masked attention numerator is
        B'_ij = adj_ij * exp(0.4*|f1_i + f2_j| + 0.6*f2_j)
    and, using sum_j A_ij = 1 to pull bias out of the values,
        P[k, i]  = sum_j x[j, k] * B'^T[j, i]      (PE, lhsT = x tiles)
        den[i]   = sum_j B'^T[j, i]                (PE, lhsT = ones)
        hT[f, i] = W^T @ (P + x_i^T * q_i) ;  h_i = hT[:, i]/(den_i + q_i) + b
    where q_i = (1 - adj_ii) * exp(0.4*|s_ii| + 0.6*f2_i) is the forced
    self-loop term for rows without one.

The host ships adj^T per core (pre-swizzled [p, t, i] f16 -- exact for a 0/1
matrix) so the device never transposes the big matrix; the j-contraction runs
directly over [j-partition, i-free] tiles.
"""

import numpy as np

N = 8192
F = 128
NCORES = 8
R = N // NCORES          # rows per core (1024)
TJ = N // 128            # j-tiles (64)
ISUB = R // 128          # core-row subtiles (8)
CH = 8                   # adj chunks
TPC = TJ // CH           # j-tiles per chunk (8)

_CACHE = {}
TRACE = False            # set True (e.g. from test.py) to capture an NTFF profile
LAST_EXEC_NS = None      # exec time from the last traced run
LAST_RESULTS = None      # full BassKernelResults of the last run


def _build_nc():
    import concourse.bass as bass
    import concourse.mybir as mybir
    import concourse.tile as tile
    from concourse import masks

    f32 = mybir.dt.float32
    f16 = mybir.dt.float16
    i16 = mybir.dt.int16
    i32 = mybir.dt.int32
    Alu = mybir.AluOpType
    Act = mybir.ActivationFunctionType

    nc = bass.Bass("TRN2", target_bir_lowering=False, debug=False,
                   num_devices=NCORES)

    adjs_d = nc.dram_tensor("adjs", [128, TJ, R], f16, kind="ExternalInput").ap()
    x16n_d = nc.dram_tensor("x16n", [128, TJ, F], f16, kind="ExternalInput").ap()
    xT16_d = nc.dram_tensor("xT16", [128, TJ, 128], f16, kind="ExternalInput").ap()
    xcT_d = nc.dram_tensor("xcT16", [128, ISUB, 128], f16, kind="ExternalInput").ap()
    W_d = nc.dram_tensor("weight", [128, 128], f32, kind="ExternalInput").ap()
    bp_d = nc.dram_tensor("bp", [128, 4], f32, kind="ExternalInput").ap()
    adjd_d = nc.dram_tensor("adjd", [128, ISUB], f32, kind="ExternalInput").ap()
    out_d = nc.dram_tensor("out", [R, F], f32, kind="ExternalOutput").ap()

    with tile.TileContext(nc) as tc:
        with tc.tile_pool(name="const", bufs=1) as cp, \
             tc.tile_pool(name="mmps", bufs=1, space="PSUM") as mmps, \
             tc.tile_pool(name="ppsA", bufs=1, space="PSUM") as ppsA, \
             tc.tile_pool(name="ppsB", bufs=2, space="PSUM") as ppsB:
            # ---------------- psum layout (8 banks exactly) ----------------
            P0 = mmps.tile([128, 512], f32, name="P0")
            P1 = mmps.tile([128, 512], f32, name="P1")
            d0 = mmps.tile([1, 512], f32, name="d0")
            d1 = mmps.tile([1, 512], f32, name="d1")
            smA = ppsA.tile([128, 512], f32, name="smA")
            smB = ppsA.tile([128, 160], f32, name="smB")

            def bc_ps():
                return ppsB.tile([128, 512], f32, tag="bc", name="bc")

            # ---------------- constants + inputs ----------------
            ones = cp.tile([1, 128], f32)
            nc.vector.memset(ones[:], 1.0)
            ones16 = cp.tile([128, 1], f16)
            nc.vector.memset(ones16[:], 1.0)
            ident = cp.tile([128, 128], f32)
            masks.make_identity(nc, ident[:])
            scr = cp.tile([1, 16], f32)     # DVE scratch for DMA-lane touches

            W_sb = cp.tile([128, 128], f32)
            nc.sync.dma_start(W_sb[:], W_d)
            bp = cp.tile([128, 4], f32)
            nc.sync.dma_start(bp[:], bp_d)
            adjd = cp.tile([128, ISUB], f32)
            nc.sync.dma_start(adjd[:], adjd_d)
            xT16 = cp.tile([128, TJ, 128], f16)
            nc.sync.dma_start(xT16[:], xT16_d)
            x16 = cp.tile([128, TJ, F], f16)
            nc.sync.dma_start(x16[:], x16n_d)
            xcT = cp.tile([128, ISUB, 128], f16)
            nc.sync.dma_start(xcT[:], xcT_d)

            # Observation chain: one tiny PE matmul reading each DMA-loaded
            # tensor, WAW-chained into one psum slot.  Later PE instructions
            # get a free same-engine ordering edge, so no matmul carries more
            # than one cross-engine sync-wait (walrus LDW fits one wait).
            obs = smB[0:1, 144:145]
            ob = nc.tensor.matmul(obs, ident[:, 0:1], ident[:, 0:1],
                                  start=True, stop=True)
            for src in (W_sb[:, 0:1], bp[:, 0:1], adjd[:, 0:1],
                        xT16[:, 0, 0:1], x16[:, 0, 0:1], xcT[:, 0, 0:1]):
                nxt = nc.tensor.matmul(obs, src, src, start=True, stop=True)
                tile.add_dep_helper(nxt.ins, ob.ins, sync=False, reason="obs")
                ob = nxt

            def _pe(inst):
                tile.add_dep_helper(inst.ins, ob.ins, sync=False, reason="obs")
                return inst

            # ---------------- prep ----------------
            # W^T (for Wphi), then Wphi = W @ phi so f = x @ Wphi
            _pe(nc.tensor.transpose(smA[:, 0:128], W_sb[:], ident[:]))
            WT = cp.tile([128, 128], f32)
            nc.scalar.copy(WT[:], smA[:, 0:128])
            _pe(nc.tensor.matmul(smA[:, 128:130], WT[:], bp[:, 1:3],
                                 start=True, stop=True))
            Wphi16 = cp.tile([128, 2], f16)
            nc.scalar.copy(Wphi16[:], smA[:, 128:130])

            # c1 = b@phi1, c2 = b@phi2 (bias-fold constants), broadcasts
            _pe(nc.tensor.matmul(smA[0:1, 132:134], bp[:, 0:1], bp[:, 1:3],
                                 start=True, stop=True))
            crow = cp.tile([1, 2], f32)
            nc.scalar.copy(crow[:], smA[0:1, 132:134])
            c12 = cp.tile([1, 1], f32)
            nc.vector.tensor_tensor(c12[:], crow[0:1, 0:1], crow[0:1, 1:2],
                                    op=Alu.add)
            _pe(nc.tensor.matmul(smA[:, 136:137], ones[:], c12[:],
                                 start=True, stop=True))
            c12b = cp.tile([128, 1], f32)
            nc.scalar.copy(c12b[:], smA[:, 136:137])
            _pe(nc.tensor.matmul(smA[:, 140:141], ones[:], crow[0:1, 1:2],
                                 start=True, stop=True))
            c2b06 = cp.tile([128, 1], f32)
            nc.scalar.activation(c2b06[:], smA[:, 140:141], Act.Copy, scale=0.6)
            bias2 = cp.tile([128, 1], f32)
            nc.scalar.copy(bias2[:], bp[:, 0:1])

            # f1/f2 projections, column form: smB[:, 2t:2t+2] = x_tile @ Wphi
            for t in range(TJ):
                _pe(nc.tensor.matmul(smB[:, 2 * t:2 * t + 2], xT16[:, t, :],
                                     Wphi16[:], start=True, stop=True))
            # core-row copies (program is SPMD-identical; core rows come from
            # the per-core xcT input): smB[:, 128+2tt : 128+2tt+2]
            for t in range(ISUB):
                _pe(nc.tensor.matmul(smB[:, 128 + 2 * t:128 + 2 * t + 2],
                                     xcT[:, t, :], Wphi16[:],
                                     start=True, stop=True))
            f2c = cp.tile([128, TJ], f32)
            nc.scalar.copy(f2c[:], smB[:, 1:129:2])
            wb = cp.tile([128, TJ], f32)    # ACT bias: 0.6*f2_j (= log w_j)
            nc.scalar.activation(wb[:], smB[:, 1:129:2], Act.Identity,
                                 scale=0.6, bias=c2b06[:])

            # f1 at core rows, row form -> broadcast over partitions (+c1+c2)
            f1r_ps = (bc_ps(), bc_ps())
            for g in range(2):
                _pe(nc.tensor.matmul(
                    f1r_ps[g][0:1, :], Wphi16[:, 0:1],
                    xcT[:, 4 * g:4 * g + 4, :].rearrange("p a b -> p (a b)"),
                    start=True, stop=True))
            f1row = cp.tile([1, R], f32)
            nc.scalar.copy(f1row[0:1, 0:512], f1r_ps[0][0:1, :])
            nc.scalar.copy(f1row[0:1, 512:1024], f1r_ps[1][0:1, :])
            nc.vector.memset(f1r_ps[0][0:1, 0:1], 0.0)   # DVE touch: WAR absorber
            nc.vector.memset(f1r_ps[1][0:1, 0:1], 0.0)
            f1b_ps = (bc_ps(), bc_ps())
            for g in range(2):
                _pe(nc.tensor.matmul(f1b_ps[g][:], ones[:],
                                     f1row[0:1, 512 * g:512 * (g + 1)],
                                     start=True, stop=True))
            f1b = cp.tile([128, R], f16)
            nc.scalar.activation(f1b[:, 0:512], f1b_ps[0][:], Act.Identity,
                                 bias=c12b[:])
            nc.scalar.activation(f1b[:, 512:1024], f1b_ps[1][:], Act.Identity,
                                 bias=c12b[:])
            nc.vector.memset(f1b_ps[0][0:1, 0:1], 0.0)
            nc.vector.memset(f1b_ps[1][0:1, 0:1], 0.0)

            # diagonal (self-loop) correction q, column form then row form
            fcore = cp.tile([128, 2 * ISUB], f32)
            nc.scalar.copy(fcore[:], smB[:, 128:144])
            sd = cp.tile([128, ISUB], f32)
            nc.vector.tensor_tensor(sd[:], fcore[:, 0:16:2], fcore[:, 1:16:2],
                                    op=Alu.add)
            sd2 = cp.tile([128, ISUB], f32)
            nc.vector.tensor_scalar(sd2[:], sd[:], c12b[:], None, op0=Alu.add)
            absd = cp.tile([128, ISUB], f32)
            nc.vector.tensor_scalar(absd[:].bitcast(i32), sd2[:].bitcast(i32),
                                    0x7FFFFFFF, None, op0=Alu.bitwise_and)
            Gd = cp.tile([128, ISUB], f32)
            nc.scalar.activation(Gd[:], absd[:], Act.Exp, scale=0.4)
            wd = cp.tile([128, ISUB], f32)
            nc.scalar.activation(wd[:], fcore[:, 1:16:2], Act.Exp,
                                 scale=0.6, bias=c2b06[:])
            mneg = cp.tile([128, ISUB], f32)
            nc.scalar.activation(mneg[:], adjd[:], Act.Identity,
                                 scale=-1.0, bias=1.0)
            qGw = cp.tile([128, ISUB], f32)
            nc.vector.tensor_tensor(qGw[:], Gd[:], wd[:], op=Alu.mult)
            qn = cp.tile([128, ISUB], f32)
            nc.vector.tensor_tensor(qn[:], qGw[:], mneg[:], op=Alu.mult)

            q_ps = (bc_ps(), bc_ps())
            for t in range(ISUB):
                _pe(nc.tensor.matmul(q_ps[t // 4][0:1, 128 * (t % 4):128 * (t % 4) + 128],
                                     qn[:, t:t + 1], ident[:],
                                     start=True, stop=True))
            q_row = cp.tile([1, R], f32)
            nc.scalar.copy(q_row[0:1, 0:512], q_ps[0][0:1, :])
            nc.scalar.copy(q_row[0:1, 512:1024], q_ps[1][0:1, :])
            nc.vector.memset(q_ps[0][0:1, 0:1], 0.0)
            nc.vector.memset(q_ps[1][0:1, 0:1], 0.0)
            qb_ps = (bc_ps(), bc_ps())
            for g in range(2):
                _pe(nc.tensor.matmul(qb_ps[g][:], ones[:],
                                     q_row[0:1, 512 * g:512 * (g + 1)],
                                     start=True, stop=True))
            qb16 = cp.tile([128, R], f16)
            nc.scalar.copy(qb16[:, 0:512], qb_ps[0][:])
            nc.scalar.copy(qb16[:, 512:1024], qb_ps[1][:])
            nc.vector.memset(qb_ps[0][0:1, 0:1], 0.0)
            nc.vector.memset(qb_ps[1][0:1, 0:1], 0.0)

            # ---------------- main loop ----------------
            with tc.tile_pool(name="adjp", bufs=3) as adjp, \
                 tc.tile_pool(name="absp", bufs=3) as absp, \
                 tc.tile_pool(name="gp", bufs=3) as gp, \
                 tc.tile_pool(name="btp", bufs=4) as btp:
                for ch in range(CH):
                    adjq = adjp.tile([128, TPC, R], f16, tag="adj")
                    nc.sync.dma_start(adjq[:], adjs_d[:, ch * TPC:(ch + 1) * TPC, :])
                    # DVE observes this chunk's DMA once, so the mults below
                    # don't each carry a DMA-lane wait.
                    nc.vector.tensor_copy(scr[0:1, ch:ch + 1],
                                          adjq[0:1, 0, 0:1])
                    for tt in range(TPC):
                        t = ch * TPC + tt
                        first = t == 0
                        last = t == TJ - 1
                        s16 = absp.tile([128, R], f16, tag="s")
                        nc.vector.tensor_scalar(s16[:], f1b[:],
                                                f2c[:, t:t + 1], None,
                                                op0=Alu.add)
                        absS = absp.tile([128, R], f16, tag="abs")
                        nc.vector.tensor_scalar(absS[:].bitcast(i16),
                                                s16[:].bitcast(i16),
                                                0x7FFF, None,
                                                op0=Alu.bitwise_and)
                        g16 = gp.tile([128, R], f16, tag="g")
                        nc.scalar.activation(g16[:], absS[:], Act.Exp,
                                             scale=0.4, bias=wb[:, t:t + 1])
                        bt = btp.tile([128, R], f16, tag="bt")
                        # mask multiply: ~2/3 on GpSimd (otherwise idle), rest
                        # on DVE, which paces the loop
                        eng = nc.vector if t % 3 == 0 else nc.gpsimd
                        eng.tensor_tensor(bt[:], g16[:], adjq[:, tt, :],
                                          op=Alu.mult)
                        mm = nc.tensor.matmul(P0[:], x16[:, t, :], bt[:, 0:512],
                                              start=first, stop=last)
                        if first:
                            tile.add_dep_helper(mm.ins, ob.ins, sync=False,
                                                reason="obs")
                        nc.tensor.matmul(P1[:], x16[:, t, :], bt[:, 512:1024],
                                         start=first, stop=last)
                        nc.tensor.matmul(d0[:], ones16[:], bt[:, 0:512],
                                         start=first, stop=last)
                        nc.tensor.matmul(d1[:], ones16[:], bt[:, 512:1024],
                                         start=first, stop=last)

            # ---------------- epilogue ----------------
            with tc.tile_pool(name="ep", bufs=1) as ep:
                # den + q, reciprocal broadcast over partitions
                dt = ep.tile([1, R], f32)
                nc.vector.tensor_tensor(dt[0:1, 0:512], d0[:],
                                        q_row[0:1, 0:512], op=Alu.add)
                nc.vector.tensor_tensor(dt[0:1, 512:1024], d1[:],
                                        q_row[0:1, 512:1024], op=Alu.add)
                recb = ep.tile([128, R], f32)
                dtb_ps = (bc_ps(), bc_ps())
                for g in range(2):
                    _pe(nc.tensor.matmul(dtb_ps[g][:], ones[:],
                                         dt[0:1, 512 * g:512 * (g + 1)],
                                         start=True, stop=True))
                    nc.vector.reciprocal(recb[:, 512 * g:512 * (g + 1)],
                                         dtb_ps[g][:])

                # numerator self-loop correction in P-space, then hT = W^T @ Pc
                u = ep.tile([128, R], f32)
                nc.vector.tensor_tensor(u[:], xcT[:].rearrange("p a b -> p (a b)"),
                                        qb16[:], op=Alu.mult)
                Pc = ep.tile([128, R], f32)
                nc.vector.tensor_tensor(Pc[:, 0:512], P0[:], u[:, 0:512],
                                        op=Alu.add)
                nc.vector.tensor_tensor(Pc[:, 512:1024], P1[:], u[:, 512:1024],
                                        op=Alu.add)
                hT_ps = (bc_ps(), bc_ps())
                for g in range(2):
                    _pe(nc.tensor.matmul(hT_ps[g][:], W_sb[:],
                                         Pc[:, 512 * g:512 * (g + 1)],
                                         start=True, stop=True))
                hn = ep.tile([128, R], f32)
                nc.vector.tensor_tensor(hn[:, 0:512], hT_ps[0][:],
                                        recb[:, 0:512], op=Alu.mult)
                nc.vector.tensor_tensor(hn[:, 512:1024], hT_ps[1][:],
                                        recb[:, 512:1024], op=Alu.mult)
                ho = ep.tile([128, R], f32)
                nc.scalar.activation(ho[:], hn[:], Act.Identity, bias=bias2[:])

                # transpose [f, i] -> [i, f] and store
                hout = ep.tile([128, ISUB, 128], f32)
                for t in range(ISUB):
                    sl = smA[:, 128 * (t % 4):128 * (t % 4) + 128]
                    _pe(nc.tensor.transpose(sl, ho[:, 128 * t:128 * (t + 1)],
                                            ident[:]))
                    nc.vector.tensor_copy(hout[:, t, :], sl)
                nc.sync.dma_start(out_d.rearrange("(a p) f -> p a f", p=128),
                                  hout[:])

    # Walrus fits at most one sync-wait per instruction; Tile emits more.
    # Run bacc's splitter (extra waits move onto EventSemaphore insts).
    from concourse.bass import _bass_rust
    _bass_rust.generate_event_semaphores(nc)
    return nc


def kernel(adj, input, weight, bias, phi):
    """Full inputs in, full output out. Shards row-wise across 8 NeuronCores."""
    adj = np.ascontiguousarray(np.asarray(adj, dtype=np.float32))
    x = np.ascontiguousarray(np.asarray(input, dtype=np.float32))
    W = np.ascontiguousarray(np.asarray(weight, dtype=np.float32))
    b = np.ascontiguousarray(np.asarray(bias, dtype=np.float32))
    phi = np.ascontiguousarray(np.asarray(phi, dtype=np.float32))

    if not _CACHE.get("use_fallback"):
        try:
            return _kernel_bass(adj, x, W, b, phi)
        except Exception:
            import traceback
            traceback.print_exc()
            _CACHE["use_fallback"] = True
    return _kernel_jax_fallback(adj, x, W, b, phi)


def _kernel_bass(adj, x, W, b, phi):
    from concourse.bass_utils import run_bass_kernel_spmd

    if "nc" not in _CACHE:
        _CACHE["nc"] = _build_nc()
    nc = _CACHE["nc"]

    # adjs[c, p, t, il] = adj[c*R + il, t*128 + p]  (transposed + swizzled, f16)
    adjs = np.ascontiguousarray(
        adj.reshape(NCORES, R, TJ, 128).transpose(0, 3, 2, 1)).astype(np.float16)
    xT = np.ascontiguousarray(x.T).astype(np.float16)          # [128, 8192]
    x16n = np.ascontiguousarray(
        x.reshape(TJ, 128, F).transpose(1, 0, 2)).astype(np.float16)
    diag = np.ascontiguousarray(np.diagonal(adj)).astype(np.float32)
    bp = np.ascontiguousarray(
        np.stack([b, phi[:F, 0], phi[F:, 0], np.zeros_like(b)], axis=1)
    ).astype(np.float32)

    in_maps = []
    for c in range(NCORES):
        r0 = c * R
        in_maps.append({
            "adjs": adjs[c],
            "x16n": x16n,
            "xT16": xT.reshape(128, TJ, 128),
            "xcT16": np.ascontiguousarray(
                xT[:, r0:r0 + R]).reshape(128, ISUB, 128),
            "weight": W,
            "bp": bp,
            "adjd": np.ascontiguousarray(
                diag[r0:r0 + R].reshape(ISUB, 128).T),
        })

    res = run_bass_kernel_spmd(nc, in_maps, core_ids=list(range(NCORES)),
                               trace=TRACE)
    global LAST_EXEC_NS, LAST_RESULTS
    LAST_RESULTS = res
    LAST_EXEC_NS = res.exec_time_ns
    parts = [res.results[c]["out"] for c in range(NCORES)]
    return np.concatenate(parts, axis=0).astype(np.float32)


def _kernel_jax_fallback(adj, x, W, b, phi):
    """Device fallback (sharded jax on the 8 NeuronCores) if the Bass path
    fails to compile/run in this environment."""
    import jax
    import jax.numpy as jnp
    from jax import lax
    from jax.sharding import Mesh, PartitionSpec, NamedSharding

    devs = jax.devices()[:NCORES]
    mesh = Mesh(np.asarray(devs), ("i",))
    row = NamedSharding(mesh, PartitionSpec("i", None))
    rep = NamedSharding(mesh, PartitionSpec())

    @jax.jit
    def f(adj_s, x_r, W_r, b_r, phi_r):
        xp = x_r @ W_r + b_r
        f1 = xp @ phi_r[:F]                      # [N, 1]
        f2 = xp @ phi_r[F:]                      # [N, 1]
        w = jnp.exp(jnp.float32(0.6) * f2[:, 0])  # [N]
        ri = lax.broadcasted_iota(jnp.int32, (N, N), 0)
        ci = lax.broadcasted_iota(jnp.int32, (N, N), 1)
        m = (adj_s > 0) | (ri == ci)
        G = jnp.exp(jnp.float32(0.4) * jnp.abs(f1 + f2.T))
        B = jnp.where(m, G * w[None, :], jnp.float32(0.0)).astype(jnp.float16)
        xpa = jnp.concatenate([xp, jnp.ones((N, 1), jnp.float32)],
                              axis=1).astype(jnp.float16)
        num = (B @ xpa).astype(jnp.float32)      # [N/8, F+1]
        return num[:, :F] / num[:, F:F + 1]

    args = (jax.device_put(adj, row), jax.device_put(x, rep),
            jax.device_put(W, rep), jax.device_put(b, rep),
            jax.device_put(phi, rep))
    out = f(*args)
    out.block_until_ready()
    if TRACE:
        import time
        global LAST_EXEC_NS
        reps = 5
        t0 = time.perf_counter()
        for _ in range(reps):
            out = f(*args)
        out.block_until_ready()
        LAST_EXEC_NS = int((time.perf_counter() - t0) / reps * 1e9)
    return np.asarray(out).astype(np.float32)


# revision 17
# speedup vs baseline: 106.8106x; 1.7065x over previous
"""GAT layer (nn_GAT_57543971832576) Bass/Tile kernel for 8 Trainium2 NeuronCores.

Math (reference):
    x' = x @ W + bias
    S_ij = leaky_relu(f1_i + f2_j, 0.2),  f1 = x'@phi1, f2 = x'@phi2
    A = softmax_j(where(adj+I > 0, S, -1e9))
    h = A @ x'

Reformulation used on device (core owns rows i = [c*1024, (c+1)*1024)):
    leaky_relu(s, 0.2) = 0.6*s + 0.4*|s|; softmax rows are invariant to the
    per-row shift 0.6*f1_i, so the masked attention numerator is
        B'_ij = adj_ij * exp(0.4*|f1_i + f2_j| + 0.6*f2_j)
    and, using sum_j A_ij = 1 to pull bias out of the values,
        P[k, i]  = sum_j x[j, k] * B'^T[j, i]      (PE, lhsT = x tiles)
        den[i]   = sum_j B'^T[j, i]                (PE, lhsT = ones)
        hT[f, i] = W^T @ (P + x_i^T * q_i) ;  h_i = hT[:, i]/(den_i + q_i) + b
    where q_i = (1 - adj_ii) * exp(0.4*|s_ii| + 0.6*f2_i) is the forced
    self-loop term for rows without one.

The host ships adj^T per core (pre-swizzled [p, t, i] f16 -- exact for a 0/1
matrix) so the device never transposes the big matrix; the j-contraction runs
directly over [j-partition, i-free] tiles.
"""

import numpy as np

N = 8192
F = 128
NCORES = 8
R = N // NCORES          # rows per core (1024)
TJ = N // 128            # j-tiles (64)
ISUB = R // 128          # core-row subtiles (8)
CH = 8                   # adj chunks
TPC = TJ // CH           # j-tiles per chunk (8)

_CACHE = {}
TRACE = False            # set True (e.g. from test.py) to capture an NTFF profile
LAST_EXEC_NS = None      # exec time from the last traced run
LAST_RESULTS = None      # full BassKernelResults of the last run


def _build_nc():
    import concourse.bass as bass
    import concourse.mybir as mybir
    import concourse.tile as tile
    from concourse import masks

    f32 = mybir.dt.float32
    f16 = mybir.dt.float16
    i16 = mybir.dt.int16
    i32 = mybir.dt.int32
    Alu = mybir.AluOpType
    Act = mybir.ActivationFunctionType

    nc = bass.Bass("TRN2", target_bir_lowering=False, debug=False,
                   num_devices=NCORES)

    adjs_d = nc.dram_tensor("adjs", [128, TJ, R], f16, kind="ExternalInput").ap()
    x16n_d = nc.dram_tensor("x16n", [128, TJ, F], f16, kind="ExternalInput").ap()
    xT16_d = nc.dram_tensor("xT16", [128, TJ, 128], f16, kind="ExternalInput").ap()
    xcT_d = nc.dram_tensor("xcT16", [128, ISUB, 128], f16, kind="ExternalInput").ap()
    W_d = nc.dram_tensor("weight", [128, 128], f32, kind="ExternalInput").ap()
    bp_d = nc.dram_tensor("bp", [128, 4], f32, kind="ExternalInput").ap()
    adjd_d = nc.dram_tensor("adjd", [128, ISUB], f32, kind="ExternalInput").ap()
    out_d = nc.dram_tensor("out", [R, F], f32, kind="ExternalOutput").ap()

    with tile.TileContext(nc) as tc:
        with tc.tile_pool(name="const", bufs=1) as cp, \
             tc.tile_pool(name="mmps", bufs=1, space="PSUM") as mmps, \
             tc.tile_pool(name="ppsA", bufs=1, space="PSUM") as ppsA, \
             tc.tile_pool(name="ppsB", bufs=2, space="PSUM") as ppsB:
            # ---------------- psum layout (8 banks exactly) ----------------
            P0 = mmps.tile([128, 512], f32, name="P0")
            P1 = mmps.tile([128, 512], f32, name="P1")
            d0 = mmps.tile([1, 512], f32, name="d0")
            d1 = mmps.tile([1, 512], f32, name="d1")
            smA = ppsA.tile([128, 512], f32, name="smA")
            smB = ppsA.tile([128, 160], f32, name="smB")

            def bc_ps():
                return ppsB.tile([128, 512], f32, tag="bc", name="bc")

            # ---------------- constants + inputs ----------------
            ones = cp.tile([1, 128], f32)
            nc.vector.memset(ones[:], 1.0)
            ones16 = cp.tile([128, 1], f16)
            nc.vector.memset(ones16[:], 1.0)
            ident = cp.tile([128, 128], f32)
            masks.make_identity(nc, ident[:])
            scr = cp.tile([1, 16], f32)     # DVE scratch for DMA-lane touches

            W_sb = cp.tile([128, 128], f32)
            nc.sync.dma_start(W_sb[:], W_d)
            bp = cp.tile([128, 4], f32)
            nc.sync.dma_start(bp[:], bp_d)
            adjd = cp.tile([128, ISUB], f32)
            nc.sync.dma_start(adjd[:], adjd_d)
            xT16 = cp.tile([128, TJ, 128], f16)
            nc.sync.dma_start(xT16[:], xT16_d)
            x16 = cp.tile([128, TJ, F], f16)
            nc.sync.dma_start(x16[:], x16n_d)
            xcT = cp.tile([128, ISUB, 128], f16)
            nc.sync.dma_start(xcT[:], xcT_d)

            # Observation chain: one tiny PE matmul reading each DMA-loaded
            # tensor, WAW-chained into one psum slot.  Later PE instructions
            # get a free same-engine ordering edge, so no matmul carries more
            # than one cross-engine sync-wait (walrus LDW fits one wait).
            obs = smB[0:1, 144:145]
            ob = nc.tensor.matmul(obs, ident[:, 0:1], ident[:, 0:1],
                                  start=True, stop=True)
            for src in (W_sb[:, 0:1], bp[:, 0:1], adjd[:, 0:1],
                        xT16[:, 0, 0:1], x16[:, 0, 0:1], xcT[:, 0, 0:1]):
                nxt = nc.tensor.matmul(obs, src, src, start=True, stop=True)
                tile.add_dep_helper(nxt.ins, ob.ins, sync=False, reason="obs")
                ob = nxt

            def _pe(inst):
                tile.add_dep_helper(inst.ins, ob.ins, sync=False, reason="obs")
                return inst

            # ---------------- prep ----------------
            # W^T (for Wphi), then Wphi = W @ phi so f = x @ Wphi
            _pe(nc.tensor.transpose(smA[:, 0:128], W_sb[:], ident[:]))
            WT = cp.tile([128, 128], f32)
            nc.scalar.copy(WT[:], smA[:, 0:128])
            _pe(nc.tensor.matmul(smA[:, 128:130], WT[:], bp[:, 1:3],
                                 start=True, stop=True))
            Wphi16 = cp.tile([128, 2], f16)
            nc.scalar.copy(Wphi16[:], smA[:, 128:130])

            # c1 = b@phi1, c2 = b@phi2 (bias-fold constants), broadcasts
            _pe(nc.tensor.matmul(smA[0:1, 132:134], bp[:, 0:1], bp[:, 1:3],
                                 start=True, stop=True))
            crow = cp.tile([1, 2], f32)
            nc.scalar.copy(crow[:], smA[0:1, 132:134])
            c12 = cp.tile([1, 1], f32)
            nc.vector.tensor_tensor(c12[:], crow[0:1, 0:1], crow[0:1, 1:2],
                                    op=Alu.add)
            _pe(nc.tensor.matmul(smA[:, 136:137], ones[:], c12[:],
                                 start=True, stop=True))
            c12b = cp.tile([128, 1], f32)
            nc.scalar.copy(c12b[:], smA[:, 136:137])
            _pe(nc.tensor.matmul(smA[:, 140:141], ones[:], crow[0:1, 1:2],
                                 start=True, stop=True))
            c2b06 = cp.tile([128, 1], f32)
            nc.scalar.activation(c2b06[:], smA[:, 140:141], Act.Copy, scale=0.6)
            bias2 = cp.tile([128, 1], f32)
            nc.scalar.copy(bias2[:], bp[:, 0:1])

            # f1/f2 projections, column form: smB[:, 2t:2t+2] = x_tile @ Wphi
            for t in range(TJ):
                _pe(nc.tensor.matmul(smB[:, 2 * t:2 * t + 2], xT16[:, t, :],
                                     Wphi16[:], start=True, stop=True))
            # core-row copies (program is SPMD-identical; core rows come from
            # the per-core xcT input): smB[:, 128+2tt : 128+2tt+2]
            for t in range(ISUB):
                _pe(nc.tensor.matmul(smB[:, 128 + 2 * t:128 + 2 * t + 2],
                                     xcT[:, t, :], Wphi16[:],
                                     start=True, stop=True))
            f2c = cp.tile([128, TJ], f32)
            nc.scalar.copy(f2c[:], smB[:, 1:129:2])
            wb = cp.tile([128, TJ], f32)    # ACT bias: 0.6*f2_j (= log w_j)
            nc.scalar.activation(wb[:], smB[:, 1:129:2], Act.Identity,
                                 scale=0.6, bias=c2b06[:])

            # f1 at core rows, row form -> broadcast over partitions (+c1+c2)
            f1r_ps = (bc_ps(), bc_ps())
            for g in range(2):
                _pe(nc.tensor.matmul(
                    f1r_ps[g][0:1, :], Wphi16[:, 0:1],
                    xcT[:, 4 * g:4 * g + 4, :].rearrange("p a b -> p (a b)"),
                    start=True, stop=True))
            f1row = cp.tile([1, R], f32)
            nc.scalar.copy(f1row[0:1, 0:512], f1r_ps[0][0:1, :])
            nc.scalar.copy(f1row[0:1, 512:1024], f1r_ps[1][0:1, :])
            nc.vector.memset(f1r_ps[0][0:1, 0:1], 0.0)   # DVE touch: WAR absorber
            nc.vector.memset(f1r_ps[1][0:1, 0:1], 0.0)
            f1b_ps = (bc_ps(), bc_ps())
            for g in range(2):
                _pe(nc.tensor.matmul(f1b_ps[g][:], ones[:],
                                     f1row[0:1, 512 * g:512 * (g + 1)],
                                     start=True, stop=True))
            f1b = cp.tile([128, R], f16)
            nc.scalar.activation(f1b[:, 0:512], f1b_ps[0][:], Act.Identity,
                                 bias=c12b[:])
            nc.scalar.activation(f1b[:, 512:1024], f1b_ps[1][:], Act.Identity,
                                 bias=c12b[:])
            nc.vector.memset(f1b_ps[0][0:1, 0:1], 0.0)
            nc.vector.memset(f1b_ps[1][0:1, 0:1], 0.0)

            # diagonal (self-loop) correction q, column form then row form
            fcore = cp.tile([128, 2 * ISUB], f32)
            nc.scalar.copy(fcore[:], smB[:, 128:144])
            sd = cp.tile([128, ISUB], f32)
            nc.vector.tensor_tensor(sd[:], fcore[:, 0:16:2], fcore[:, 1:16:2],
                                    op=Alu.add)
            sd2 = cp.tile([128, ISUB], f32)
            nc.vector.tensor_scalar(sd2[:], sd[:], c12b[:], None, op0=Alu.add)
            absd = cp.tile([128, ISUB], f32)
            nc.vector.tensor_scalar(absd[:].bitcast(i32), sd2[:].bitcast(i32),
                                    0x7FFFFFFF, None, op0=Alu.bitwise_and)
            Gd = cp.tile([128, ISUB], f32)
            nc.scalar.activation(Gd[:], absd[:], Act.Exp, scale=0.4)
            wd = cp.tile([128, ISUB], f32)
            nc.scalar.activation(wd[:], fcore[:, 1:16:2], Act.Exp,
                                 scale=0.6, bias=c2b06[:])
            mneg = cp.tile([128, ISUB], f32)
            nc.scalar.activation(mneg[:], adjd[:], Act.Identity,
                                 scale=-1.0, bias=1.0)
            qGw = cp.tile([128, ISUB], f32)
            nc.vector.tensor_tensor(qGw[:], Gd[:], wd[:], op=Alu.mult)
            qn = cp.tile([128, ISUB], f32)
            nc.vector.tensor_tensor(qn[:], qGw[:], mneg[:], op=Alu.mult)

            q_ps = (bc_ps(), bc_ps())
            for t in range(ISUB):
                _pe(nc.tensor.matmul(q_ps[t // 4][0:1, 128 * (t % 4):128 * (t % 4) + 128],
                                     qn[:, t:t + 1], ident[:],
                                     start=True, stop=True))
            q_row = cp.tile([1, R], f32)
            nc.scalar.copy(q_row[0:1, 0:512], q_ps[0][0:1, :])
            nc.scalar.copy(q_row[0:1, 512:1024], q_ps[1][0:1, :])
            nc.vector.memset(q_ps[0][0:1, 0:1], 0.0)
            nc.vector.memset(q_ps[1][0:1, 0:1], 0.0)
            qb_ps = (bc_ps(), bc_ps())
            for g in range(2):
                _pe(nc.tensor.matmul(qb_ps[g][:], ones[:],
                                     q_row[0:1, 512 * g:512 * (g + 1)],
                                     start=True, stop=True))
            qb16 = cp.tile([128, R], f16)
            nc.scalar.copy(qb16[:, 0:512], qb_ps[0][:])
            nc.scalar.copy(qb16[:, 512:1024], qb_ps[1][:])
            nc.vector.memset(qb_ps[0][0:1, 0:1], 0.0)
            nc.vector.memset(qb_ps[1][0:1, 0:1], 0.0)

            # ---------------- main loop ----------------
            with tc.tile_pool(name="adjp", bufs=3) as adjp, \
                 tc.tile_pool(name="absp", bufs=3) as absp, \
                 tc.tile_pool(name="gp", bufs=3) as gp, \
                 tc.tile_pool(name="btp", bufs=4) as btp:
                for ch in range(CH):
                    adjq = adjp.tile([128, TPC, R], f16, tag="adj")
                    nc.sync.dma_start(adjq[:], adjs_d[:, ch * TPC:(ch + 1) * TPC, :])
                    # DVE observes this chunk's DMA once, so the mults below
                    # don't each carry a DMA-lane wait.
                    nc.vector.tensor_copy(scr[0:1, ch:ch + 1],
                                          adjq[0:1, 0, 0:1])
                    for pq in range(TPC // 2):
                        t0 = ch * TPC + 2 * pq
                        # tensor_scalar needs the per-tile f2_j scalar, but
                        # abs and the mask multiply are tile-independent —
                        # batch those over tile pairs (halves DVE overhead)
                        s16p = absp.tile([128, 2, R], f16, tag="s")
                        for k in range(2):
                            nc.vector.tensor_scalar(
                                s16p[:, k, :], f1b[:],
                                f2c[:, t0 + k:t0 + k + 1], None, op0=Alu.add)
                        absp_t = absp.tile([128, 2, R], f16, tag="abs")
                        nc.vector.tensor_scalar(
                            absp_t[:].rearrange("p a b -> p (a b)").bitcast(i16),
                            s16p[:].rearrange("p a b -> p (a b)").bitcast(i16),
                            0x7FFF, None, op0=Alu.bitwise_and)
                        g16p = gp.tile([128, 2, R], f16, tag="g")
                        for k in range(2):
                            nc.scalar.activation(
                                g16p[:, k, :], absp_t[:, k, :], Act.Exp,
                                scale=0.4, bias=wb[:, t0 + k:t0 + k + 1])
                        btpr = btp.tile([128, 2, R], f16, tag="bt")
                        nc.vector.tensor_tensor(
                            btpr[:].rearrange("p a b -> p (a b)"),
                            g16p[:].rearrange("p a b -> p (a b)"),
                            adjq[:, 2 * pq:2 * pq + 2, :].rearrange(
                                "p a b -> p (a b)"),
                            op=Alu.mult)
                        for k in range(2):
                            t = t0 + k
                            first = t == 0
                            last = t == TJ - 1
                            mm = nc.tensor.matmul(P0[:], x16[:, t, :],
                                                  btpr[:, k, 0:512],
                                                  start=first, stop=last)
                            if first:
                                tile.add_dep_helper(mm.ins, ob.ins, sync=False,
                                                    reason="obs")
                            nc.tensor.matmul(P1[:], x16[:, t, :],
                                             btpr[:, k, 512:1024],
                                             start=first, stop=last)
                            nc.tensor.matmul(d0[:], ones16[:],
                                             btpr[:, k, 0:512],
                                             start=first, stop=last)
                            nc.tensor.matmul(d1[:], ones16[:],
                                             btpr[:, k, 512:1024],
                                             start=first, stop=last)

            # ---------------- epilogue ----------------
            with tc.tile_pool(name="ep", bufs=1) as ep:
                # den + q, reciprocal broadcast over partitions
                dt = ep.tile([1, R], f32)
                nc.vector.tensor_tensor(dt[0:1, 0:512], d0[:],
                                        q_row[0:1, 0:512], op=Alu.add)
                nc.vector.tensor_tensor(dt[0:1, 512:1024], d1[:],
                                        q_row[0:1, 512:1024], op=Alu.add)
                recb = ep.tile([128, R], f32)
                dtb_ps = (bc_ps(), bc_ps())
                for g in range(2):
                    _pe(nc.tensor.matmul(dtb_ps[g][:], ones[:],
                                         dt[0:1, 512 * g:512 * (g + 1)],
                                         start=True, stop=True))
                    nc.vector.reciprocal(recb[:, 512 * g:512 * (g + 1)],
                                         dtb_ps[g][:])

                # numerator self-loop correction in P-space, then hT = W^T @ Pc
                u = ep.tile([128, R], f32)
                nc.vector.tensor_tensor(u[:], xcT[:].rearrange("p a b -> p (a b)"),
                                        qb16[:], op=Alu.mult)
                Pc = ep.tile([128, R], f32)
                nc.vector.tensor_tensor(Pc[:, 0:512], P0[:], u[:, 0:512],
                                        op=Alu.add)
                nc.vector.tensor_tensor(Pc[:, 512:1024], P1[:], u[:, 512:1024],
                                        op=Alu.add)
                hT_ps = (bc_ps(), bc_ps())
                for g in range(2):
                    _pe(nc.tensor.matmul(hT_ps[g][:], W_sb[:],
                                         Pc[:, 512 * g:512 * (g + 1)],
                                         start=True, stop=True))
                hn = ep.tile([128, R], f32)
                nc.vector.tensor_tensor(hn[:, 0:512], hT_ps[0][:],
                                        recb[:, 0:512], op=Alu.mult)
                nc.vector.tensor_tensor(hn[:, 512:1024], hT_ps[1][:],
                                        recb[:, 512:1024], op=Alu.mult)
                ho = ep.tile([128, R], f32)
                nc.scalar.activation(ho[:], hn[:], Act.Identity, bias=bias2[:])

                # transpose [f, i] -> [i, f] and store
                hout = ep.tile([128, ISUB, 128], f32)
                for t in range(ISUB):
                    sl = smA[:, 128 * (t % 4):128 * (t % 4) + 128]
                    _pe(nc.tensor.transpose(sl, ho[:, 128 * t:128 * (t + 1)],
                                            ident[:]))
                    nc.vector.tensor_copy(hout[:, t, :], sl)
                nc.sync.dma_start(out_d.rearrange("(a p) f -> p a f", p=128),
                                  hout[:])

    # Walrus fits at most one sync-wait per instruction; Tile emits more.
    # Run bacc's splitter (extra waits move onto EventSemaphore insts).
    from concourse.bass import _bass_rust
    _bass_rust.generate_event_semaphores(nc)
    return nc


def kernel(adj, input, weight, bias, phi):
    """Full inputs in, full output out. Shards row-wise across 8 NeuronCores."""
    adj = np.ascontiguousarray(np.asarray(adj, dtype=np.float32))
    x = np.ascontiguousarray(np.asarray(input, dtype=np.float32))
    W = np.ascontiguousarray(np.asarray(weight, dtype=np.float32))
    b = np.ascontiguousarray(np.asarray(bias, dtype=np.float32))
    phi = np.ascontiguousarray(np.asarray(phi, dtype=np.float32))

    if not _CACHE.get("use_fallback"):
        try:
            return _kernel_bass(adj, x, W, b, phi)
        except Exception:
            import traceback
            traceback.print_exc()
            _CACHE["use_fallback"] = True
    return _kernel_jax_fallback(adj, x, W, b, phi)


def _kernel_bass(adj, x, W, b, phi):
    from concourse.bass_utils import run_bass_kernel_spmd

    if "nc" not in _CACHE:
        _CACHE["nc"] = _build_nc()
    nc = _CACHE["nc"]

    # adjs[c, p, t, il] = adj[c*R + il, t*128 + p]  (transposed + swizzled, f16)
    adjs = np.ascontiguousarray(
        adj.reshape(NCORES, R, TJ, 128).transpose(0, 3, 2, 1)).astype(np.float16)
    xT = np.ascontiguousarray(x.T).astype(np.float16)          # [128, 8192]
    x16n = np.ascontiguousarray(
        x.reshape(TJ, 128, F).transpose(1, 0, 2)).astype(np.float16)
    diag = np.ascontiguousarray(np.diagonal(adj)).astype(np.float32)
    bp = np.ascontiguousarray(
        np.stack([b, phi[:F, 0], phi[F:, 0], np.zeros_like(b)], axis=1)
    ).astype(np.float32)

    in_maps = []
    for c in range(NCORES):
        r0 = c * R
        in_maps.append({
            "adjs": adjs[c],
            "x16n": x16n,
            "xT16": xT.reshape(128, TJ, 128),
            "xcT16": np.ascontiguousarray(
                xT[:, r0:r0 + R]).reshape(128, ISUB, 128),
            "weight": W,
            "bp": bp,
            "adjd": np.ascontiguousarray(
                diag[r0:r0 + R].reshape(ISUB, 128).T),
        })

    res = run_bass_kernel_spmd(nc, in_maps, core_ids=list(range(NCORES)),
                               trace=TRACE)
    global LAST_EXEC_NS, LAST_RESULTS
    LAST_RESULTS = res
    LAST_EXEC_NS = res.exec_time_ns
    parts = [res.results[c]["out"] for c in range(NCORES)]
    return np.concatenate(parts, axis=0).astype(np.float32)


def _kernel_jax_fallback(adj, x, W, b, phi):
    """Device fallback (sharded jax on the 8 NeuronCores) if the Bass path
    fails to compile/run in this environment."""
    import jax
    import jax.numpy as jnp
    from jax import lax
    from jax.sharding import Mesh, PartitionSpec, NamedSharding

    devs = jax.devices()[:NCORES]
    mesh = Mesh(np.asarray(devs), ("i",))
    row = NamedSharding(mesh, PartitionSpec("i", None))
    rep = NamedSharding(mesh, PartitionSpec())

    @jax.jit
    def f(adj_s, x_r, W_r, b_r, phi_r):
        xp = x_r @ W_r + b_r
        f1 = xp @ phi_r[:F]                      # [N, 1]
        f2 = xp @ phi_r[F:]                      # [N, 1]
        w = jnp.exp(jnp.float32(0.6) * f2[:, 0])  # [N]
        ri = lax.broadcasted_iota(jnp.int32, (N, N), 0)
        ci = lax.broadcasted_iota(jnp.int32, (N, N), 1)
        m = (adj_s > 0) | (ri == ci)
        G = jnp.exp(jnp.float32(0.4) * jnp.abs(f1 + f2.T))
        B = jnp.where(m, G * w[None, :], jnp.float32(0.0)).astype(jnp.float16)
        xpa = jnp.concatenate([xp, jnp.ones((N, 1), jnp.float32)],
                              axis=1).astype(jnp.float16)
        num = (B @ xpa).astype(jnp.float32)      # [N/8, F+1]
        return num[:, :F] / num[:, F:F + 1]

    args = (jax.device_put(adj, row), jax.device_put(x, rep),
            jax.device_put(W, rep), jax.device_put(b, rep),
            jax.device_put(phi, rep))
    out = f(*args)
    out.block_until_ready()
    if TRACE:
        import time
        global LAST_EXEC_NS
        reps = 5
        t0 = time.perf_counter()
        for _ in range(reps):
            out = f(*args)
        out.block_until_ready()
        LAST_EXEC_NS = int((time.perf_counter() - t0) / reps * 1e9)
    return np.asarray(out).astype(np.float32)


# revision 24
# speedup vs baseline: 108.6779x; 1.0175x over previous
"""GAT layer (nn_GAT_57543971832576) Bass/Tile kernel for 8 Trainium2 NeuronCores.

Math (reference):
    x' = x @ W + bias
    S_ij = leaky_relu(f1_i + f2_j, 0.2),  f1 = x'@phi1, f2 = x'@phi2
    A = softmax_j(where(adj+I > 0, S, -1e9))
    h = A @ x'

Reformulation used on device (core owns rows i = [c*1024, (c+1)*1024)):
    leaky_relu(s, 0.2) = 0.6*s + 0.4*|s|; softmax rows are invariant to the
    per-row shift 0.6*f1_i, so the masked attention numerator is
        B'_ij = adj_ij * exp(0.4*|f1_i + f2_j| + 0.6*f2_j)
    and, using sum_j A_ij = 1 to pull bias out of the values,
        P[k, i]  = sum_j x[j, k] * B'^T[j, i]      (PE, lhsT = x tiles)
        den[i]   = sum_j B'^T[j, i]                (PE, lhsT = ones)
        hT[f, i] = W^T @ (P + x_i^T * q_i) ;  h_i = hT[:, i]/(den_i + q_i) + b
    where q_i = (1 - adj_ii) * exp(0.4*|s_ii| + 0.6*f2_i) is the forced
    self-loop term for rows without one.

The host ships adj^T per core (pre-swizzled [p, t, i] f16 -- exact for a 0/1
matrix) so the device never transposes the big matrix; the j-contraction runs
directly over [j-partition, i-free] tiles.
"""

import numpy as np

N = 8192
F = 128
NCORES = 8
R = N // NCORES          # rows per core (1024)
TJ = N // 128            # j-tiles (64)
ISUB = R // 128          # core-row subtiles (8)
CH = 8                   # adj chunks
TPC = TJ // CH           # j-tiles per chunk (8)

_CACHE = {}
TRACE = False            # set True (e.g. from test.py) to capture an NTFF profile
LAST_EXEC_NS = None      # exec time from the last traced run
LAST_RESULTS = None      # full BassKernelResults of the last run


def _build_nc():
    import concourse.bass as bass
    import concourse.mybir as mybir
    import concourse.tile as tile
    from concourse import masks

    f32 = mybir.dt.float32
    f16 = mybir.dt.float16
    i16 = mybir.dt.int16
    i32 = mybir.dt.int32
    Alu = mybir.AluOpType
    Act = mybir.ActivationFunctionType

    nc = bass.Bass("TRN2", target_bir_lowering=False, debug=False,
                   num_devices=NCORES)

    adjs_d = nc.dram_tensor("adjs", [128, TJ, R], f16, kind="ExternalInput").ap()
    x16n_d = nc.dram_tensor("x16n", [128, TJ, F], f16, kind="ExternalInput").ap()
    xT16_d = nc.dram_tensor("xT16", [128, TJ, 128], f16, kind="ExternalInput").ap()
    xcT_d = nc.dram_tensor("xcT16", [128, ISUB, 128], f16, kind="ExternalInput").ap()
    W_d = nc.dram_tensor("weight", [128, 128], f32, kind="ExternalInput").ap()
    bp_d = nc.dram_tensor("bp", [128, 4], f32, kind="ExternalInput").ap()
    adjd_d = nc.dram_tensor("adjd", [128, ISUB], f32, kind="ExternalInput").ap()
    out_d = nc.dram_tensor("out", [R, F], f32, kind="ExternalOutput").ap()

    with tile.TileContext(nc) as tc:
        with tc.tile_pool(name="const", bufs=1) as cp, \
             tc.tile_pool(name="mmps", bufs=1, space="PSUM") as mmps, \
             tc.tile_pool(name="ppsA", bufs=1, space="PSUM") as ppsA, \
             tc.tile_pool(name="ppsB", bufs=2, space="PSUM") as ppsB:
            # ---------------- psum layout (8 banks exactly) ----------------
            P0 = mmps.tile([128, 512], f32, name="P0")
            P1 = mmps.tile([128, 512], f32, name="P1")
            d0 = mmps.tile([1, 512], f32, name="d0")
            d1 = mmps.tile([1, 512], f32, name="d1")
            smA = ppsA.tile([128, 512], f32, name="smA")
            smB = ppsA.tile([128, 160], f32, name="smB")

            def bc_ps():
                return ppsB.tile([128, 512], f32, tag="bc", name="bc")

            # ---------------- constants + inputs ----------------
            ones = cp.tile([1, 128], f32)
            nc.vector.memset(ones[:], 1.0)
            ones16 = cp.tile([128, 1], f16)
            nc.vector.memset(ones16[:], 1.0)
            ident = cp.tile([128, 128], f32)
            masks.make_identity(nc, ident[:])
            scr = cp.tile([1, 16], f32)     # DVE scratch for DMA-lane touches

            W_sb = cp.tile([128, 128], f32)
            nc.sync.dma_start(W_sb[:], W_d)
            bp = cp.tile([128, 4], f32)
            nc.sync.dma_start(bp[:], bp_d)
            adjd = cp.tile([128, ISUB], f32)
            nc.sync.dma_start(adjd[:], adjd_d)
            xT16 = cp.tile([128, TJ, 128], f16)
            nc.sync.dma_start(xT16[:], xT16_d)
            x16 = cp.tile([128, TJ, F], f16)
            nc.sync.dma_start(x16[:], x16n_d)
            xcT = cp.tile([128, ISUB, 128], f16)
            nc.sync.dma_start(xcT[:], xcT_d)

            # Observation chain: one tiny PE matmul reading each DMA-loaded
            # tensor, WAW-chained into one psum slot.  Later PE instructions
            # get a free same-engine ordering edge, so no matmul carries more
            # than one cross-engine sync-wait (walrus LDW fits one wait).
            obs = smB[0:1, 144:145]
            ob = nc.tensor.matmul(obs, ident[:, 0:1], ident[:, 0:1],
                                  start=True, stop=True)
            for src in (W_sb[:, 0:1], bp[:, 0:1], adjd[:, 0:1],
                        xT16[:, 0, 0:1], x16[:, 0, 0:1], xcT[:, 0, 0:1]):
                nxt = nc.tensor.matmul(obs, src, src, start=True, stop=True)
                tile.add_dep_helper(nxt.ins, ob.ins, sync=False, reason="obs")
                ob = nxt

            def _pe(inst):
                tile.add_dep_helper(inst.ins, ob.ins, sync=False, reason="obs")
                return inst

            # ---------------- prep ----------------
            # W^T (for Wphi), then Wphi = W @ phi so f = x @ Wphi
            _pe(nc.tensor.transpose(smA[:, 0:128], W_sb[:], ident[:]))
            WT = cp.tile([128, 128], f32)
            nc.scalar.copy(WT[:], smA[:, 0:128])
            _pe(nc.tensor.matmul(smA[:, 128:130], WT[:], bp[:, 1:3],
                                 start=True, stop=True))
            Wphi16 = cp.tile([128, 2], f16)
            nc.scalar.copy(Wphi16[:], smA[:, 128:130])

            # c1 = b@phi1, c2 = b@phi2 (bias-fold constants), broadcasts
            _pe(nc.tensor.matmul(smA[0:1, 132:134], bp[:, 0:1], bp[:, 1:3],
                                 start=True, stop=True))
            crow = cp.tile([1, 2], f32)
            nc.scalar.copy(crow[:], smA[0:1, 132:134])
            c12 = cp.tile([1, 1], f32)
            nc.vector.tensor_tensor(c12[:], crow[0:1, 0:1], crow[0:1, 1:2],
                                    op=Alu.add)
            _pe(nc.tensor.matmul(smA[:, 136:137], ones[:], c12[:],
                                 start=True, stop=True))
            c12b = cp.tile([128, 1], f32)
            nc.scalar.copy(c12b[:], smA[:, 136:137])
            _pe(nc.tensor.matmul(smA[:, 140:141], ones[:], crow[0:1, 1:2],
                                 start=True, stop=True))
            c2b06 = cp.tile([128, 1], f32)
            nc.scalar.activation(c2b06[:], smA[:, 140:141], Act.Copy, scale=0.6)
            bias2 = cp.tile([128, 1], f32)
            nc.scalar.copy(bias2[:], bp[:, 0:1])

            # f1/f2 projections, column form: smB[:, 2t:2t+2] = x_tile @ Wphi
            for t in range(TJ):
                _pe(nc.tensor.matmul(smB[:, 2 * t:2 * t + 2], xT16[:, t, :],
                                     Wphi16[:], start=True, stop=True))
            # core-row copies (program is SPMD-identical; core rows come from
            # the per-core xcT input): smB[:, 128+2tt : 128+2tt+2]
            for t in range(ISUB):
                _pe(nc.tensor.matmul(smB[:, 128 + 2 * t:128 + 2 * t + 2],
                                     xcT[:, t, :], Wphi16[:],
                                     start=True, stop=True))
            f2c = cp.tile([128, TJ], f32)
            nc.scalar.copy(f2c[:], smB[:, 1:129:2])
            wb = cp.tile([128, TJ], f32)    # ACT bias: 0.6*f2_j (= log w_j)
            nc.scalar.activation(wb[:], smB[:, 1:129:2], Act.Identity,
                                 scale=0.6, bias=c2b06[:])

            # f1 at core rows, row form -> broadcast over partitions (+c1+c2)
            f1r_ps = (bc_ps(), bc_ps())
            for g in range(2):
                _pe(nc.tensor.matmul(
                    f1r_ps[g][0:1, :], Wphi16[:, 0:1],
                    xcT[:, 4 * g:4 * g + 4, :].rearrange("p a b -> p (a b)"),
                    start=True, stop=True))
            f1row = cp.tile([1, R], f32)
            nc.scalar.copy(f1row[0:1, 0:512], f1r_ps[0][0:1, :])
            nc.scalar.copy(f1row[0:1, 512:1024], f1r_ps[1][0:1, :])
            nc.vector.memset(f1r_ps[0][0:1, 0:1], 0.0)   # DVE touch: WAR absorber
            nc.vector.memset(f1r_ps[1][0:1, 0:1], 0.0)
            f1b_ps = (bc_ps(), bc_ps())
            for g in range(2):
                _pe(nc.tensor.matmul(f1b_ps[g][:], ones[:],
                                     f1row[0:1, 512 * g:512 * (g + 1)],
                                     start=True, stop=True))
            f1b = cp.tile([128, R], f16)
            nc.scalar.activation(f1b[:, 0:512], f1b_ps[0][:], Act.Identity,
                                 bias=c12b[:])
            nc.scalar.activation(f1b[:, 512:1024], f1b_ps[1][:], Act.Identity,
                                 bias=c12b[:])
            nc.vector.memset(f1b_ps[0][0:1, 0:1], 0.0)
            nc.vector.memset(f1b_ps[1][0:1, 0:1], 0.0)

            # diagonal (self-loop) correction q, column form then row form
            fcore = cp.tile([128, 2 * ISUB], f32)
            nc.scalar.copy(fcore[:], smB[:, 128:144])
            sd = cp.tile([128, ISUB], f32)
            nc.vector.tensor_tensor(sd[:], fcore[:, 0:16:2], fcore[:, 1:16:2],
                                    op=Alu.add)
            sd2 = cp.tile([128, ISUB], f32)
            nc.vector.tensor_scalar(sd2[:], sd[:], c12b[:], None, op0=Alu.add)
            absd = cp.tile([128, ISUB], f32)
            nc.vector.tensor_scalar(absd[:].bitcast(i32), sd2[:].bitcast(i32),
                                    0x7FFFFFFF, None, op0=Alu.bitwise_and)
            Gd = cp.tile([128, ISUB], f32)
            nc.scalar.activation(Gd[:], absd[:], Act.Exp, scale=0.4)
            wd = cp.tile([128, ISUB], f32)
            nc.scalar.activation(wd[:], fcore[:, 1:16:2], Act.Exp,
                                 scale=0.6, bias=c2b06[:])
            mneg = cp.tile([128, ISUB], f32)
            nc.scalar.activation(mneg[:], adjd[:], Act.Identity,
                                 scale=-1.0, bias=1.0)
            qGw = cp.tile([128, ISUB], f32)
            nc.vector.tensor_tensor(qGw[:], Gd[:], wd[:], op=Alu.mult)
            qn = cp.tile([128, ISUB], f32)
            nc.vector.tensor_tensor(qn[:], qGw[:], mneg[:], op=Alu.mult)

            q_ps = (bc_ps(), bc_ps())
            for t in range(ISUB):
                _pe(nc.tensor.matmul(q_ps[t // 4][0:1, 128 * (t % 4):128 * (t % 4) + 128],
                                     qn[:, t:t + 1], ident[:],
                                     start=True, stop=True))
            q_row = cp.tile([1, R], f32)
            nc.scalar.copy(q_row[0:1, 0:512], q_ps[0][0:1, :])
            nc.scalar.copy(q_row[0:1, 512:1024], q_ps[1][0:1, :])
            nc.vector.memset(q_ps[0][0:1, 0:1], 0.0)
            nc.vector.memset(q_ps[1][0:1, 0:1], 0.0)
            qb_ps = (bc_ps(), bc_ps())
            for g in range(2):
                _pe(nc.tensor.matmul(qb_ps[g][:], ones[:],
                                     q_row[0:1, 512 * g:512 * (g + 1)],
                                     start=True, stop=True))
            qb16 = cp.tile([128, R], f16)
            nc.scalar.copy(qb16[:, 0:512], qb_ps[0][:])
            nc.scalar.copy(qb16[:, 512:1024], qb_ps[1][:])
            nc.vector.memset(qb_ps[0][0:1, 0:1], 0.0)
            nc.vector.memset(qb_ps[1][0:1, 0:1], 0.0)

            # ---------------- main loop ----------------
            with tc.tile_pool(name="adjp", bufs=3) as adjp, \
                 tc.tile_pool(name="absp", bufs=4) as absp, \
                 tc.tile_pool(name="gp", bufs=3) as gp, \
                 tc.tile_pool(name="btp", bufs=4) as btp:
                for ch in range(CH):
                    adjq = adjp.tile([128, TPC, R], f16, tag="adj")
                    nc.sync.dma_start(adjq[:], adjs_d[:, ch * TPC:(ch + 1) * TPC, :])
                    # DVE observes this chunk's DMA once, so the mults below
                    # don't each carry a DMA-lane wait.
                    nc.vector.tensor_copy(scr[0:1, ch:ch + 1],
                                          adjq[0:1, 0, 0:1])
                    for pq in range(TPC // 2):
                        t0 = ch * TPC + 2 * pq
                        # tensor_scalar needs the per-tile f2_j scalar, but
                        # abs and the mask multiply are tile-independent —
                        # batch those over tile pairs (halves DVE overhead)
                        s16p = absp.tile([128, 2, R], f16, tag="s")
                        for k in range(2):
                            nc.vector.tensor_scalar(
                                s16p[:, k, :], f1b[:],
                                f2c[:, t0 + k:t0 + k + 1], None, op0=Alu.add)
                        absp_t = absp.tile([128, 2, R], f16, tag="abs")
                        nc.vector.tensor_scalar(
                            absp_t[:].rearrange("p a b -> p (a b)").bitcast(i16),
                            s16p[:].rearrange("p a b -> p (a b)").bitcast(i16),
                            0x7FFF, None, op0=Alu.bitwise_and)
                        g16p = gp.tile([128, 2, R], f16, tag="g")
                        for k in range(2):
                            nc.scalar.activation(
                                g16p[:, k, :], absp_t[:, k, :], Act.Exp,
                                scale=0.4, bias=wb[:, t0 + k:t0 + k + 1])
                        btpr = btp.tile([128, 2, R], f16, tag="bt")
                        nc.vector.tensor_tensor(
                            btpr[:].rearrange("p a b -> p (a b)"),
                            g16p[:].rearrange("p a b -> p (a b)"),
                            adjq[:, 2 * pq:2 * pq + 2, :].rearrange(
                                "p a b -> p (a b)"),
                            op=Alu.mult)
                        for k in range(2):
                            t = t0 + k
                            first = t == 0
                            last = t == TJ - 1
                            mm = nc.tensor.matmul(P0[:], x16[:, t, :],
                                                  btpr[:, k, 0:512],
                                                  start=first, stop=last)
                            if first:
                                tile.add_dep_helper(mm.ins, ob.ins, sync=False,
                                                    reason="obs")
                            nc.tensor.matmul(P1[:], x16[:, t, :],
                                             btpr[:, k, 512:1024],
                                             start=first, stop=last)
                            nc.tensor.matmul(d0[:], ones16[:],
                                             btpr[:, k, 0:512],
                                             start=first, stop=last)
                            nc.tensor.matmul(d1[:], ones16[:],
                                             btpr[:, k, 512:1024],
                                             start=first, stop=last)

            # ---------------- epilogue ----------------
            with tc.tile_pool(name="ep", bufs=1) as ep:
                # den + q, reciprocal broadcast over partitions
                dt = ep.tile([1, R], f32)
                nc.vector.tensor_tensor(dt[0:1, 0:512], d0[:],
                                        q_row[0:1, 0:512], op=Alu.add)
                nc.vector.tensor_tensor(dt[0:1, 512:1024], d1[:],
                                        q_row[0:1, 512:1024], op=Alu.add)
                # 1/den via exp(-ln(den)) on ACT: Ln and Exp share one
                # activation-table set, and this beats DVE's iterative divide
                recb = ep.tile([128, R], f32)
                lnd = ep.tile([128, R], f32)
                dtb_ps = (bc_ps(), bc_ps())
                for g in range(2):
                    _pe(nc.tensor.matmul(dtb_ps[g][:], ones[:],
                                         dt[0:1, 512 * g:512 * (g + 1)],
                                         start=True, stop=True))
                    nc.scalar.activation(lnd[:, 512 * g:512 * (g + 1)],
                                         dtb_ps[g][:], Act.Ln)
                nc.scalar.activation(recb[:], lnd[:], Act.Exp, scale=-1.0)

                # numerator self-loop correction in P-space, then hT = W^T @ Pc
                u = ep.tile([128, R], f32)
                nc.vector.tensor_tensor(u[:], xcT[:].rearrange("p a b -> p (a b)"),
                                        qb16[:], op=Alu.mult)
                Pc = ep.tile([128, R], f32)
                nc.vector.tensor_tensor(Pc[:, 0:512], P0[:], u[:, 0:512],
                                        op=Alu.add)
                nc.vector.tensor_tensor(Pc[:, 512:1024], P1[:], u[:, 512:1024],
                                        op=Alu.add)
                hT_ps = (bc_ps(), bc_ps())
                for g in range(2):
                    _pe(nc.tensor.matmul(hT_ps[g][:], W_sb[:],
                                         Pc[:, 512 * g:512 * (g + 1)],
                                         start=True, stop=True))
                hn = ep.tile([128, R], f32)
                nc.vector.tensor_tensor(hn[:, 0:512], hT_ps[0][:],
                                        recb[:, 0:512], op=Alu.mult)
                nc.vector.tensor_tensor(hn[:, 512:1024], hT_ps[1][:],
                                        recb[:, 512:1024], op=Alu.mult)
                ho = ep.tile([128, R], f32)
                nc.scalar.activation(ho[:], hn[:], Act.Identity, bias=bias2[:])

                # transpose [f, i] -> [i, f] and store
                hout = ep.tile([128, ISUB, 128], f32)
                for t in range(ISUB):
                    sl = smA[:, 128 * (t % 4):128 * (t % 4) + 128]
                    _pe(nc.tensor.transpose(sl, ho[:, 128 * t:128 * (t + 1)],
                                            ident[:]))
                    nc.vector.tensor_copy(hout[:, t, :], sl)
                nc.sync.dma_start(out_d.rearrange("(a p) f -> p a f", p=128),
                                  hout[:])

    # Walrus fits at most one sync-wait per instruction; Tile emits more.
    # Run bacc's splitter (extra waits move onto EventSemaphore insts).
    from concourse.bass import _bass_rust
    _bass_rust.generate_event_semaphores(nc)
    return nc


def kernel(adj, input, weight, bias, phi):
    """Full inputs in, full output out. Shards row-wise across 8 NeuronCores."""
    adj = np.ascontiguousarray(np.asarray(adj, dtype=np.float32))
    x = np.ascontiguousarray(np.asarray(input, dtype=np.float32))
    W = np.ascontiguousarray(np.asarray(weight, dtype=np.float32))
    b = np.ascontiguousarray(np.asarray(bias, dtype=np.float32))
    phi = np.ascontiguousarray(np.asarray(phi, dtype=np.float32))

    if not _CACHE.get("use_fallback"):
        try:
            return _kernel_bass(adj, x, W, b, phi)
        except Exception:
            import traceback
            traceback.print_exc()
            _CACHE["use_fallback"] = True
    return _kernel_jax_fallback(adj, x, W, b, phi)


def _kernel_bass(adj, x, W, b, phi):
    from concourse.bass_utils import run_bass_kernel_spmd

    if "nc" not in _CACHE:
        _CACHE["nc"] = _build_nc()
    nc = _CACHE["nc"]

    # adjs[c, p, t, il] = adj[c*R + il, t*128 + p]  (transposed + swizzled, f16)
    adjs = np.ascontiguousarray(
        adj.reshape(NCORES, R, TJ, 128).transpose(0, 3, 2, 1)).astype(np.float16)
    xT = np.ascontiguousarray(x.T).astype(np.float16)          # [128, 8192]
    x16n = np.ascontiguousarray(
        x.reshape(TJ, 128, F).transpose(1, 0, 2)).astype(np.float16)
    diag = np.ascontiguousarray(np.diagonal(adj)).astype(np.float32)
    bp = np.ascontiguousarray(
        np.stack([b, phi[:F, 0], phi[F:, 0], np.zeros_like(b)], axis=1)
    ).astype(np.float32)

    in_maps = []
    for c in range(NCORES):
        r0 = c * R
        in_maps.append({
            "adjs": adjs[c],
            "x16n": x16n,
            "xT16": xT.reshape(128, TJ, 128),
            "xcT16": np.ascontiguousarray(
                xT[:, r0:r0 + R]).reshape(128, ISUB, 128),
            "weight": W,
            "bp": bp,
            "adjd": np.ascontiguousarray(
                diag[r0:r0 + R].reshape(ISUB, 128).T),
        })

    res = run_bass_kernel_spmd(nc, in_maps, core_ids=list(range(NCORES)),
                               trace=TRACE)
    global LAST_EXEC_NS, LAST_RESULTS
    LAST_RESULTS = res
    LAST_EXEC_NS = res.exec_time_ns
    parts = [res.results[c]["out"] for c in range(NCORES)]
    return np.concatenate(parts, axis=0).astype(np.float32)


def _kernel_jax_fallback(adj, x, W, b, phi):
    """Device fallback (sharded jax on the 8 NeuronCores) if the Bass path
    fails to compile/run in this environment."""
    import jax
    import jax.numpy as jnp
    from jax import lax
    from jax.sharding import Mesh, PartitionSpec, NamedSharding

    devs = jax.devices()[:NCORES]
    mesh = Mesh(np.asarray(devs), ("i",))
    row = NamedSharding(mesh, PartitionSpec("i", None))
    rep = NamedSharding(mesh, PartitionSpec())

    @jax.jit
    def f(adj_s, x_r, W_r, b_r, phi_r):
        xp = x_r @ W_r + b_r
        f1 = xp @ phi_r[:F]                      # [N, 1]
        f2 = xp @ phi_r[F:]                      # [N, 1]
        w = jnp.exp(jnp.float32(0.6) * f2[:, 0])  # [N]
        ri = lax.broadcasted_iota(jnp.int32, (N, N), 0)
        ci = lax.broadcasted_iota(jnp.int32, (N, N), 1)
        m = (adj_s > 0) | (ri == ci)
        G = jnp.exp(jnp.float32(0.4) * jnp.abs(f1 + f2.T))
        B = jnp.where(m, G * w[None, :], jnp.float32(0.0)).astype(jnp.float16)
        xpa = jnp.concatenate([xp, jnp.ones((N, 1), jnp.float32)],
                              axis=1).astype(jnp.float16)
        num = (B @ xpa).astype(jnp.float32)      # [N/8, F+1]
        return num[:, :F] / num[:, F:F + 1]

    args = (jax.device_put(adj, row), jax.device_put(x, rep),
            jax.device_put(W, rep), jax.device_put(b, rep),
            jax.device_put(phi, rep))
    out = f(*args)
    out.block_until_ready()
    if TRACE:
        import time
        global LAST_EXEC_NS
        reps = 5
        t0 = time.perf_counter()
        for _ in range(reps):
            out = f(*args)
        out.block_until_ready()
        LAST_EXEC_NS = int((time.perf_counter() - t0) / reps * 1e9)
    return np.asarray(out).astype(np.float32)
